# revision 1
# baseline (speedup 1.0000x reference)
"""Trainium2 Bass kernel for nn_DotProductAttention (softmax over QUERY axis).

reference:
    scores  = einsum("bqd,bkd->bqk", q, k) / sqrt(d)      # [B, Lq, Lk]
    weights = softmax(scores, axis=1)                     # over q (axis 1!)
    out     = einsum("bqk,bkd->bqd", weights, v)          # [B, Lq, d]

Sharding: data-parallel over batch, one batch element per NeuronCore (B=8).

Per-core algorithm (Lq=Lk=2048, d=64):
  - Stage q/k/v with the row permutation row = p*16 + t (partition-major)
    so every DMA reads/writes contiguous 4KB per partition.
  - Transpose Q,K (cast to bf16) to [d, L] layout via PE identity-matmul
    transposes (two 128x64 tiles per transpose); duplicate into partitions
    64-127 so paired k-tiles can use disjoint PE row groups concurrently.
  - For each k-tile pair (A even, B odd; 128 K-rows each):
      S_T[k, q] = (K Q^T)[k, q]   k on partitions, q on the free axis ->
      softmax over q is a free-axis op. A uses PE rows 0-63, B rows 64-127
      (tile_position row groups -> concurrent matmuls).
      exp with scale=1/sqrt(d) folded in. Softmax denominator: h=0 half
      summed on the vector engine (tensor_reduce of the bf16 E tile),
      h=1 half via activation accum_out - splits the reduction work
      across engines since ACT is the critical path. Fold 1/s into V.
      O_T[d, q] += V'^T E  accumulated in PSUM; A writes PE cols 0-63,
      B cols 64-127 -> concurrent. Explicit ordering deps keep the next
      pair's S matmuls AHEAD of this pair's O matmuls in the PE queue
      (the activation engine is the critical path and its next exp gates
      on those S matmuls).
  - Epilogue: sum the even/odd O_T halves into a partition-packed
    [128, 1024] buffer (q-blocks 0-7 on partitions 0-63, 8-15 on 64-127)
    so one PE transpose emits two output q-tiles; single bulk DMA out.

No max-subtraction in softmax: scores ~ N(0,1), max over 2048 ~ 4; exp
never overflows and fp32 exp is exact to ~2 ULP here.
"""

import contextlib
import os
import sys

for _p in ("/opt/trn_rl_repo", "/root/.axon_site/_ro/trn_rl_repo"):
    if os.path.isdir(_p) and _p not in sys.path:
        sys.path.append(_p)

import numpy as np

import concourse.bacc as bacc
import concourse.bass as bass
import concourse.mybir as mybir
import concourse.tile as tile
from concourse.bass_utils import run_bass_kernel_spmd
from concourse.masks import make_identity

B, LQ, LK, D = 8, 2048, 2048, 64
P = 128                  # partitions
NT = LK // P             # 16 k-tiles (and q-tiles)
NC = 4                   # 512-column chunks per 2048
F32 = mybir.dt.float32
MM_DT = mybir.dt.bfloat16


def _emit(tc: tile.TileContext, o_ap, q_ap, k_ap, v_ap):
    nc = tc.nc
    Exp = mybir.ActivationFunctionType.Exp

    with contextlib.ExitStack() as ctx:
        consts = ctx.enter_context(tc.tile_pool(name="consts", bufs=1))
        stage = ctx.enter_context(tc.tile_pool(name="stage", bufs=1))
        trbuf = ctx.enter_context(tc.tile_pool(name="trbuf", bufs=1))
        epool = ctx.enter_context(tc.tile_pool(name="epool", bufs=6))
        small = ctx.enter_context(tc.tile_pool(name="small", bufs=12))
        vpool = ctx.enter_context(tc.tile_pool(name="vpool", bufs=4))
        psum_s = ctx.enter_context(
            tc.tile_pool(name="psum_s", bufs=2, space=bass.MemorySpace.PSUM)
        )
        psum_o = ctx.enter_context(
            tc.tile_pool(name="psum_o", bufs=1, space=bass.MemorySpace.PSUM)
        )

        identity = consts.tile([P, P], MM_DT)
        make_identity(nc, identity)
        identity_f32 = consts.tile([P, P], F32)
        make_identity(nc, identity_f32)

        # ---- staged, chunked input pipeline ---------------------------
        # Row permutation: HBM row p*NT+t <-> SBUF [p, t, :]; contiguous
        # 4KB per partition per DMA. Applied identically to q, k, v and
        # the output, so the kernel is exactly equivalent.
        qt_ch = [trbuf.tile([P, 512], MM_DT, name=f"qt{c}") for c in range(NC)]
        kt_ch = [trbuf.tile([P, 512], MM_DT, name=f"kt{c}") for c in range(NC)]
        q3 = q_ap.rearrange("(p t) d -> p t d", t=NT)
        k3 = k_ap.rearrange("(p t) d -> p t d", t=NT)

        def do_chunk(name, ap3, dst, c, ce, ptag="o"):
            """DMA 4 row-tiles, cast to bf16, PE-transpose into [d, 512],
            copy into the duplicated [128, 512] chunk. ce = engine for the
            PSUM->SBUF copies (nc.vector or nc.scalar)."""
            st = stage.tile([P, 4, D], F32, tag=f"st_{name}", bufs=2,
                            name=f"st_{name}{c}")
            nc.sync.dma_start(out=st, in_=ap3[:, 4 * c:4 * c + 4, :])
            bf = stage.tile([P, 4, D], MM_DT, tag=f"bf_{name}", bufs=2,
                            name=f"bf_{name}{c}")
            nc.vector.tensor_copy(bf, st)
            pool = psum_o if ptag == "o" else psum_s
            tp_ps = pool.tile([P, 256], MM_DT, tag="o0" if ptag == "o" else ptag,
                              name=f"tp_{name}{c}")
            for j in range(2):
                # two tiles per transpose: out partitions 0-63 hold tile
                # 2j's [d, 128], partitions 64-127 tile 2j+1's
                nc.tensor.transpose(
                    tp_ps[:, j * P:(j + 1) * P], bf[:, 2 * j:2 * j + 2, :],
                    identity,
                )
            cp = nc.scalar.copy if ce is nc.scalar else nc.vector.tensor_copy
            for t in range(4):
                cp(
                    dst[0:D, t * P:(t + 1) * P],
                    tp_ps[(t % 2) * D:(t % 2 + 1) * D,
                          (t // 2) * P:(t // 2 + 1) * P],
                )
            cp(dst[D:P, :], dst[0:D, :])

        # chunks needed for the first exp go first; ACT (idle during the
        # prologue) handles their copies
        do_chunk("q", q3, qt_ch[0], 0, nc.scalar, ptag="sps")
        do_chunk("q", q3, qt_ch[1], 1, nc.vector, ptag="sps")
        do_chunk("k", k3, kt_ch[0], 0, nc.scalar, ptag="sps")
        v_stage = stage.tile([P, NT, D], F32)
        nc.sync.dma_start(out=v_stage, in_=v_ap.rearrange("(p t) d -> p t d", t=NT))

        rng = ((0, D), (D, P))  # member A: PE rows/cols 0-63, B: 64-127

        def s_matmuls(kp, h):
            """Interleaved A/B score matmuls for half h of pair kp (A on PE
            rows 0-63, B on rows 64-127 -> concurrent)."""
            s_ps2 = [
                psum_s.tile([P, 1024], F32, tag="sps", name=f"s{kp}_{h}_{m}")
                for m in range(2)
            ]
            # member-outer: A's two matmuls issue back-to-back right after
            # A's previous exp releases its PSUM slot (B's slot frees one
            # exp later and must not block A in the PE queue)
            with tc.high_priority(offset=25):
                for m in range(2):
                    kt = 2 * kp + m
                    r0, r1 = rng[m]
                    for n in range(2):
                        c = h * 2 + n
                        nc.tensor.matmul(
                            s_ps2[m][:, n * 512:(n + 1) * 512],
                            lhsT=kt_ch[kt // 4][r0:r1, (kt % 4) * P:(kt % 4 + 1) * P],
                            rhs=qt_ch[c][r0:r1, :],
                            start=True,
                            stop=True,
                        )
            return s_ps2

        # ---- main loop over k-tile pairs (software-pipelined) ---------
        # O_T accumulators, one per 512-col chunk so the epilogue can
        # start as soon as a chunk's accumulation group closes.
        # [0:64]=even-kt O_T, [64:128]=odd-kt O_T.
        o_ps = [psum_o.tile([P, 512], F32, tag=f"o{n}", name=f"ops{n}")
                for n in range(NC)]
        NP = NT // 2
        late_chunks = [("q", q3, qt_ch[2], 2), ("q", q3, qt_ch[3], 3),
                       ("k", k3, kt_ch[1], 1), ("k", k3, kt_ch[2], 2),
                       ("k", k3, kt_ch[3], 3)]
        # low scheduler priority: these feed pairs >= 1 and must not
        # crowd out the first pair's S matmuls on the PE
        with tc.high_priority(offset=-250):
            for args in late_chunks:
                do_chunk(*args, nc.vector)
        s_next = s_matmuls(0, 0)
        for kp in range(NP):
            e_tiles = [epool.tile([P, LQ], MM_DT, tag="e", name=f"e{kp}_{m}")
                       for m in range(2)]
            ssum = [[], []]
            for h in range(2):
                s_ps2 = s_next
                for m in range(2):
                    sh = small.tile([P, 1], F32, tag="shalf", bufs=64,
                                    name=f"sh{kp}_{h}_{m}")
                    nc.scalar.activation(
                        out=e_tiles[m][:, h * 1024:(h + 1) * 1024],
                        in_=s_ps2[m],
                        func=Exp,
                        scale=0.125,      # 1/sqrt(64)
                        accum_out=sh,
                    )
                    ssum[m].append(sh)
                if h == 0:
                    s_next = s_matmuls(kp, 1)
                elif kp + 1 < NP:
                    s_next = s_matmuls(kp + 1, 0)
            v_scs = []
            for m in range(2):
                kt = 2 * kp + m
                stot = small.tile([P, 1], F32, tag="stot", bufs=32,
                                  name=f"st{kp}_{m}")
                nc.vector.tensor_add(stot, ssum[m][0], ssum[m][1])
                rec = small.tile([P, 1], F32, tag="rec", bufs=32,
                                 name=f"rc{kp}_{m}")
                nc.vector.reciprocal(rec, stot)
                v_sc = vpool.tile([P, D], MM_DT, tag="vsc", bufs=8,
                                  name=f"vs{kp}_{m}")
                nc.vector.tensor_scalar_mul(v_sc, v_stage[:, kt, :], rec)
                v_scs.append(v_sc)
            # O matmuls, A/B interleaved (disjoint PE col groups)
            for n in range(NC):
                for m in range(2):
                    r0, r1 = rng[m]
                    nc.tensor.matmul(
                        o_ps[n][r0:r1, :],
                        lhsT=v_scs[m],
                        rhs=e_tiles[m][:, n * 512:(n + 1) * 512],
                        start=(kp == 0),
                        stop=(kp == NP - 1),
                    )

        # ---- epilogue: O_T = even half + odd half; [d, q] -> [q, d] ----
        # partition-packed per 512-col chunk: q-blocks 4n..4n+3 land as
        # (even blocks -> partitions 0-63, odd -> 64-127) so each PE
        # transpose of [128, 128] emits two ADJACENT output q-tiles and
        # the whole chain pipelines with the tail O matmuls chunk by chunk.
        o_pk = trbuf.tile([P, 1024], F32)
        o_out3 = o_ap.rearrange("(p t) d -> p t d", t=NT)
        for n in range(NC):
            o_hi = trbuf.tile([D, 512], F32, tag="ohi", bufs=4, name=f"oh{n}")
            nc.scalar.copy(o_hi, o_ps[n][D:P, :])
            hi3 = o_hi.rearrange("d (b c) -> d b c", c=P)
            lo3 = o_ps[n][0:D, :].rearrange("d (b c) -> d b c", c=P)
            pk3 = o_pk[:, 2 * n * P:(2 * n + 2) * P].rearrange(
                "d (b c) -> d b c", c=P)
            # even blocks (4n, 4n+2) -> partitions 0-63; odd -> 64-127
            nc.vector.tensor_add(pk3[0:D, :, :], lo3[:, 0::2, :], hi3[:, 0::2, :])
            nc.vector.tensor_add(pk3[D:P, :, :], lo3[:, 1::2, :], hi3[:, 1::2, :])
            for j in range(2):
                b = 2 * n + j
                ot_ps = psum_s.tile([P, P], F32, tag="sps", name=f"ot{b}")
                nc.tensor.transpose(
                    ot_ps, o_pk[:, b * P:(b + 1) * P], identity_f32
                )
                cp = nc.vector.tensor_copy if j == 0 else nc.scalar.copy
                out_st = stage.tile([P, 2, D], F32, tag="outst", bufs=4,
                                    name=f"ou{b}")
                cp(out_st[:, 0, :], ot_ps[:, 0:D])
                cp(out_st[:, 1, :], ot_ps[:, D:P])
                nc.sync.dma_start(
                    out=o_out3[:, 4 * n + 2 * j:4 * n + 2 * j + 2, :],
                    in_=out_st,
                )


_CACHED = {}


def _build():
    if "nc" in _CACHED:
        return _CACHED["nc"]
    nc = bacc.Bacc("TRN2", target_bir_lowering=False, debug=False)
    q = nc.dram_tensor("q", [LQ, D], F32, kind="ExternalInput")
    k = nc.dram_tensor("k", [LK, D], F32, kind="ExternalInput")
    v = nc.dram_tensor("v", [LK, D], F32, kind="ExternalInput")
    o = nc.dram_tensor("o", [LQ, D], F32, kind="ExternalOutput")
    with tile.TileContext(nc) as tc:
        _emit(tc, o[:], q[:], k[:], v[:])
    nc.finalize()
    _CACHED["nc"] = nc
    return nc


def kernel(query, key, value, _trace=False, _trace_kwargs=None):
    query = np.asarray(query, dtype=np.float32)
    key = np.asarray(key, dtype=np.float32)
    value = np.asarray(value, dtype=np.float32)
    assert query.shape == (B, LQ, D), query.shape
    nc = _build()
    in_maps = [
        {
            "q": np.ascontiguousarray(query[i]),
            "k": np.ascontiguousarray(key[i]),
            "v": np.ascontiguousarray(value[i]),
        }
        for i in range(B)
    ]
    kwargs = {}
    if _trace:
        kwargs["trace"] = True
        kwargs.update(_trace_kwargs or {})
    res = run_bass_kernel_spmd(nc, in_maps, core_ids=list(range(B)), **kwargs)
    out = np.stack([res.results[i]["o"] for i in range(B)])
    if _trace:
        return out, res
    return out


if __name__ == "__main__":
    rng = np.random.default_rng(0)
    q = rng.standard_normal((B, LQ, D), dtype=np.float32)
    k = rng.standard_normal((B, LQ, D), dtype=np.float32)
    v = rng.standard_normal((B, LQ, D), dtype=np.float32)
    o = kernel(q, k, v)
    print(o.shape, o.dtype)



# revision 8
# speedup vs baseline: 1.0967x; 1.0967x over previous
"""Trainium2 Bass kernel for nn_DotProductAttention (softmax over QUERY axis).

reference:
    scores  = einsum("bqd,bkd->bqk", q, k) / sqrt(d)      # [B, Lq, Lk]
    weights = softmax(scores, axis=1)                     # over q (axis 1!)
    out     = einsum("bqk,bkd->bqd", weights, v)          # [B, Lq, d]

Sharding: data-parallel over batch, one batch element per NeuronCore (B=8).

Host-side work is layout-only (slicing/transpose/reshape for the chosen
sharding): each core receives qk = [q_i^T ; k_i^T] stacked [128, 2048] f32
and v pre-shuffled to [128, 16, 64] (partition = k % 128).  The core
returns O^T [64, 2048] bf16 which the host transposes back.  All
arithmetic (casts, matmuls, softmax) runs on device.

Per-core algorithm (Lq=Lk=2048, d=64):
  - The scalar (ACT) engine is the hard floor: 4.2M exps must flow
    through it (~35us).  It therefore runs ONLY the 32 [128,1024] exp
    instructions; everything else lives on other engines.
  - S^T[k, q] per k-tile via PE: lhsT = K^T tile, rhs = Q^T, contract
    over d=64.  Even k-tiles (member A) use PE rows 0-63, odd (B) rows
    64-127, so consecutive matmuls overlap on disjoint quadrants.  Both
    operand layouts come straight from SBUF: QKa = [Q^T ; K^T] stacked on
    partitions (cast from the DMAed f32), QKb = [K^T ; Q^T] duplicated
    via SBUF->SBUF DMA (no PE transposes, no PSUM staging for inputs).
  - S PSUM is a 3-deep ring of [128,1024] tiles (6 banks): the S matmuls
    for round r+3 only wait on exp(r), giving the PE ~2 exp-times of
    slack so ACT never starves.
  - Softmax denominators: DVE tensor_reduce over the bf16 E slices,
    then add/reciprocal/scale-V (all DVE).  1/Z folds into V rows.
  - O^T[d, q] accumulates in PSUM over all 16 k-tiles AND both members
    (A+B sum falls out of PSUM accumulation for free).  Two [128,512]
    tiles (2 banks): q-chunks 0/2 on partitions 0-63, 1/3 on 64-127.
    Consecutive O matmuls alternate chunk column-quadrants.
  - Epilogue: 4 cast-copies (PSUM f32 -> bf16, split ACT/DVE) + 2 DMAs.

No max-subtraction in softmax: scores ~ N(0,1), max over 2048 ~ 5; exp
never overflows and fp32 exp is exact to ~2 ULP here.
"""

import contextlib
import os
import sys

for _p in ("/opt/trn_rl_repo", "/root/.axon_site/_ro/trn_rl_repo"):
    if os.path.isdir(_p) and _p not in sys.path:
        sys.path.append(_p)

import numpy as np

import concourse.bacc as bacc
import concourse.bass as bass
import concourse.mybir as mybir
import concourse.tile as tile
from concourse.bass_utils import run_bass_kernel_spmd

B, LQ, LK, D = 8, 2048, 2048, 64
P = 128                  # partitions
NT = LK // P             # 16 k-tiles
NP = NT // 2             # 8 k-tile pairs (A = even tile, B = odd tile)
NR = 4 * NP              # 32 rounds; round = one [128,1024] exp
F32 = mybir.dt.float32
BF16 = mybir.dt.bfloat16


def _emit(tc: tile.TileContext, o_ap, qk_ap, v_ap):
    nc = tc.nc
    Exp = mybir.ActivationFunctionType.Exp
    AxX = mybir.AxisListType.X
    Add = mybir.AluOpType.add

    with contextlib.ExitStack() as ctx:
        consts = ctx.enter_context(tc.tile_pool(name="consts", bufs=1))
        stage = ctx.enter_context(tc.tile_pool(name="stage", bufs=1))
        qkpool = ctx.enter_context(tc.tile_pool(name="qkpool", bufs=1))
        epool = ctx.enter_context(tc.tile_pool(name="epool", bufs=6))
        vpool = ctx.enter_context(tc.tile_pool(name="vpool", bufs=6))
        small = ctx.enter_context(tc.tile_pool(name="small", bufs=32))
        psum_s = ctx.enter_context(
            tc.tile_pool(name="psum_s", bufs=3, space=bass.MemorySpace.PSUM)
        )
        psum_o = ctx.enter_context(
            tc.tile_pool(name="psum_o", bufs=1, space=bass.MemorySpace.PSUM)
        )

        # ---- preload the Exp activation table while DMAs run ----------
        dummy = consts.tile([P, 1], F32)
        nc.gpsimd.memset(dummy, 0.0)
        dume = consts.tile([P, 1], F32)
        nc.scalar.activation(out=dume, in_=dummy, func=Exp)

        # ---- input staging --------------------------------------------
        # qk_ap [128, 2048] f32: partitions 0-63 = Q^T (d-major), 64-127
        # = K^T.  Cast to bf16 into QKa; QKb = partition-swapped copy
        # ([K^T ; Q^T]) via SBUF->SBUF DMA so member A reads its lhsT
        # (K^T) from partitions 0-63 and member B its rhs (Q^T) from
        # 64-127.
        qka = qkpool.tile([P, LQ], BF16)
        qkb = qkpool.tile([P, LQ], BF16)
        for c in range(4):
            sl = slice(512 * c, 512 * c + 512)
            st = stage.tile([P, 512], F32, tag="st", bufs=2, name=f"st{c}")
            nc.sync.dma_start(out=st, in_=qk_ap[:, sl])
            nc.vector.tensor_copy(qka[:, sl], st)
            nc.sync.dma_start(out=qkb[0:D, sl], in_=qka[D:P, sl])
            nc.sync.dma_start(out=qkb[D:P, sl], in_=qka[0:D, sl])

        # v_ap [128, 16, 64] f32, already host-shuffled so that
        # v_stage[p, t, :] = v row (t*128 + p): tile t = k-rows
        # 128t..128t+127 on partitions, ready as O-matmul lhsT.
        v_stage = stage.tile([P, NT, D], F32)
        nc.sync.dma_start(out=v_stage[:, 0:8, :], in_=v_ap[:, 0:8, :])
        nc.sync.dma_start(out=v_stage[:, 8:16, :], in_=v_ap[:, 8:16, :])

        # ---- S matmul rounds ------------------------------------------
        # round r = (pair p = r//4, half h = (r%4)//2, member m = r%2)
        # S^T tile [128 k-rows, 1024 q-cols] for k-tile 2p+m, q-half h.
        def s_round(r):
            p, h, m = r // 4, (r % 4) // 2, r % 2
            kt = 2 * p + m
            sps = psum_s.tile([P, 1024], F32, tag="s", bufs=3, name=f"sps{r}")
            if m == 0:   # A: PE rows 0-63
                lhsT = qkb[0:D, kt * P:(kt + 1) * P]
                rhs_src, r0, r1 = qka, 0, D
            else:        # B: PE rows 64-127
                lhsT = qka[D:P, kt * P:(kt + 1) * P]
                rhs_src, r0, r1 = qkb, D, P
            with tc.high_priority(offset=25):
                for n in range(2):
                    q0 = h * 1024 + n * 512
                    nc.tensor.matmul(
                        sps[:, n * 512:(n + 1) * 512],
                        lhsT=lhsT,
                        rhs=rhs_src[r0:r1, q0:q0 + 512],
                        start=True,
                        stop=True,
                    )
            return sps

        # O^T accumulators: both members accumulate into the same rows
        # (the A+B sum is free PSUM accumulation).  Chunk n = q-cols
        # [512n, 512n+512): chunks 0/1 share o_ps01 (partitions 0-63 /
        # 64-127), chunks 2/3 share o_ps23.
        o_ps01 = psum_o.tile([P, 512], F32, tag="o01", name="ops01")
        o_ps23 = psum_o.tile([P, 512], F32, tag="o23", name="ops23")

        def o_half(n):
            t = o_ps01 if n < 2 else o_ps23
            return t[0:D, :] if n % 2 == 0 else t[D:P, :]

        pend = [s_round(r) for r in range(3)]
        for p in range(NP):
            e_ab = [epool.tile([P, LQ], BF16, tag="e", name=f"e{p}_{m}")
                    for m in range(2)]
            sh = [[None, None], [None, None]]
            for h in range(2):
                for m in range(2):
                    r = 4 * p + 2 * h + m
                    sps = pend.pop(0)
                    esl = e_ab[m][:, h * 1024:(h + 1) * 1024]
                    nc.scalar.activation(
                        out=esl, in_=sps, func=Exp, scale=0.125,
                    )
                    if r + 3 < NR:
                        pend.append(s_round(r + 3))
                    shm = small.tile([P, 1], F32, tag="sh", bufs=32,
                                     name=f"sh{r}")
                    nc.vector.tensor_reduce(shm, esl, axis=AxX, op=Add)
                    sh[m][h] = shm
            v_scs = []
            for m in range(2):
                kt = 2 * p + m
                stot = small.tile([P, 1], F32, tag="stot", bufs=16,
                                  name=f"st{p}_{m}")
                nc.vector.tensor_add(stot, sh[m][0], sh[m][1])
                rec = small.tile([P, 1], F32, tag="rec", bufs=16,
                                 name=f"rc{p}_{m}")
                nc.vector.reciprocal(rec, stot)
                v_sc = vpool.tile([P, D], BF16, tag="vsc", name=f"vs{p}_{m}")
                nc.vector.tensor_scalar_mul(v_sc, v_stage[:, kt, :], rec)
                v_scs.append(v_sc)
            # 8 O matmuls per pair; consecutive ones alternate column
            # quadrants (chunk parity) so they overlap on the PE.
            for nb in range(2):          # chunk block: (0,1) then (2,3)
                for m in range(2):
                    for n in (2 * nb, 2 * nb + 1):
                        nc.tensor.matmul(
                            o_half(n),
                            lhsT=v_scs[m],
                            rhs=e_ab[m][:, n * 512:(n + 1) * 512],
                            start=(p == 0 and m == 0),
                            stop=(p == NP - 1 and m == 1),
                        )

        # ---- epilogue: cast O^T chunks to bf16, DMA out as [64, 2048] -
        # ACT is idle after the last exp; DVE still drains the last Z
        # chain.  Split the 4 cast-copies across both.
        obuf = qkpool.tile([D, LQ], BF16)
        for n in range(4):
            sl = slice(512 * n, 512 * n + 512)
            eng = nc.scalar.copy if n % 2 == 0 else nc.vector.tensor_copy
            eng(obuf[:, sl], o_half(n))
        nc.sync.dma_start(out=o_ap[:, 0:1024], in_=obuf[:, 0:1024])
        nc.sync.dma_start(out=o_ap[:, 1024:2048], in_=obuf[:, 1024:2048])


_CACHED = {}


def _build():
    if "nc" in _CACHED:
        return _CACHED["nc"]
    nc = bacc.Bacc("TRN2", target_bir_lowering=False, debug=False)
    qk = nc.dram_tensor("qk", [P, LQ], F32, kind="ExternalInput")
    v = nc.dram_tensor("v", [P, NT, D], F32, kind="ExternalInput")
    o = nc.dram_tensor("o", [D, LQ], BF16, kind="ExternalOutput")
    with tile.TileContext(nc) as tc:
        _emit(tc, o[:], qk[:], v[:])
    nc.finalize()
    _CACHED["nc"] = nc
    return nc


def kernel(query, key, value, _trace=False, _trace_kwargs=None):
    query = np.asarray(query, dtype=np.float32)
    key = np.asarray(key, dtype=np.float32)
    value = np.asarray(value, dtype=np.float32)
    assert query.shape == (B, LQ, D), query.shape
    nc = _build()
    in_maps = []
    for i in range(B):
        qk = np.empty((P, LQ), dtype=np.float32)
        qk[0:D] = query[i].T
        qk[D:P] = key[i].T
        vsh = np.ascontiguousarray(
            value[i].reshape(NT, P, D).transpose(1, 0, 2))
        in_maps.append({"qk": qk, "v": vsh})
    kwargs = {}
    if _trace:
        kwargs["trace"] = True
        kwargs.update(_trace_kwargs or {})
    res = run_bass_kernel_spmd(nc, in_maps, core_ids=list(range(B)), **kwargs)
    out = np.stack([
        np.asarray(res.results[i]["o"]).astype(np.float32).T
        for i in range(B)
    ])
    if _trace:
        return out, res
    return out


if __name__ == "__main__":
    rng = np.random.default_rng(0)
    q = rng.standard_normal((B, LQ, D), dtype=np.float32)
    k = rng.standard_normal((B, LQ, D), dtype=np.float32)
    v = rng.standard_normal((B, LQ, D), dtype=np.float32)
    o = kernel(q, k, v)
    print(o.shape, o.dtype)


# revision 11
# speedup vs baseline: 1.1250x; 1.0259x over previous
"""Trainium2 Bass kernel for nn_DotProductAttention (softmax over QUERY axis).

reference:
    scores  = einsum("bqd,bkd->bqk", q, k) / sqrt(d)      # [B, Lq, Lk]
    weights = softmax(scores, axis=1)                     # over q (axis 1!)
    out     = einsum("bqk,bkd->bqd", weights, v)          # [B, Lq, d]

Sharding: data-parallel over batch, one batch element per NeuronCore (B=8).

Host-side work is layout-only (slicing/transpose/reshape for the chosen
sharding): each core receives qk = [q_i^T ; k_i^T] stacked [128, 2048] f32
and v pre-shuffled to [128, 16, 64] (partition = k % 128).  The core
returns O^T [64, 2048] bf16 which the host transposes back.  All
arithmetic (casts, matmuls, softmax) runs on device.

Per-core algorithm (Lq=Lk=2048, d=64):
  - The scalar (ACT) engine is the hard floor: 4.2M exps must flow
    through it (~35us).  It therefore runs ONLY the 32 [128,1024] exp
    instructions; everything else lives on other engines.
  - S^T[k, q] per k-tile via PE: lhsT = K^T tile, rhs = Q^T, contract
    over d=64.  Even k-tiles (member A) use PE rows 0-63, odd (B) rows
    64-127, so consecutive matmuls overlap on disjoint quadrants.  Both
    operand layouts come straight from SBUF: QKa = [Q^T ; K^T] stacked on
    partitions (cast from the DMAed f32), QKb = [K^T ; Q^T] duplicated
    via SBUF->SBUF DMA (no PE transposes, no PSUM staging for inputs).
  - S PSUM is a 3-deep ring of [128,1024] tiles (6 banks): the S matmuls
    for round r+3 only wait on exp(r), giving the PE ~2 exp-times of
    slack so ACT never starves.
  - Softmax denominators: DVE tensor_reduce over the bf16 E slices,
    then add/reciprocal/scale-V (all DVE).  1/Z folds into V rows.
  - O^T[d, q] accumulates in PSUM over all 16 k-tiles AND both members
    (A+B sum falls out of PSUM accumulation for free).  Two [128,512]
    tiles (2 banks): q-chunks 0/2 on partitions 0-63, 1/3 on 64-127.
    Consecutive O matmuls alternate chunk column-quadrants.
  - Epilogue: 4 cast-copies (PSUM f32 -> bf16, split ACT/DVE) + 2 DMAs.

No max-subtraction in softmax: scores ~ N(0,1), max over 2048 ~ 5; exp
never overflows and fp32 exp is exact to ~2 ULP here.
"""

import contextlib
import os
import sys

for _p in ("/opt/trn_rl_repo", "/root/.axon_site/_ro/trn_rl_repo"):
    if os.path.isdir(_p) and _p not in sys.path:
        sys.path.append(_p)

import numpy as np

import concourse.bacc as bacc
import concourse.bass as bass
import concourse.mybir as mybir
import concourse.tile as tile
from concourse.bass_utils import run_bass_kernel_spmd

B, LQ, LK, D = 8, 2048, 2048, 64
P = 128                  # partitions
NT = LK // P             # 16 k-tiles
NP = NT // 2             # 8 k-tile pairs (A = even tile, B = odd tile)
NR = 4 * NP              # 32 rounds; round = one [128,1024] exp
F32 = mybir.dt.float32
BF16 = mybir.dt.bfloat16


def _emit(tc: tile.TileContext, o_ap, qk_ap, v_ap):
    nc = tc.nc
    Exp = mybir.ActivationFunctionType.Exp
    AxX = mybir.AxisListType.X
    Add = mybir.AluOpType.add

    with contextlib.ExitStack() as ctx:
        consts = ctx.enter_context(tc.tile_pool(name="consts", bufs=1))
        stage = ctx.enter_context(tc.tile_pool(name="stage", bufs=1))
        qkpool = ctx.enter_context(tc.tile_pool(name="qkpool", bufs=1))
        epool = ctx.enter_context(tc.tile_pool(name="epool", bufs=6))
        vpool = ctx.enter_context(tc.tile_pool(name="vpool", bufs=6))
        small = ctx.enter_context(tc.tile_pool(name="small", bufs=32))
        psum_s = ctx.enter_context(
            tc.tile_pool(name="psum_s", bufs=3, space=bass.MemorySpace.PSUM)
        )
        psum_o = ctx.enter_context(
            tc.tile_pool(name="psum_o", bufs=1, space=bass.MemorySpace.PSUM)
        )

        # ---- preload the Exp activation table while DMAs run ----------
        dummy = consts.tile([P, 1], F32)
        nc.gpsimd.memset(dummy, 0.0)
        dume = consts.tile([P, 1], F32)
        nc.scalar.activation(out=dume, in_=dummy, func=Exp)

        # ---- input staging --------------------------------------------
        # qk_ap [128, 2048] f32: partitions 0-63 = Q^T (d-major), 64-127
        # = K^T.  Cast to bf16 into QKa; QKb = partition-swapped copy
        # ([K^T ; Q^T]) via SBUF->SBUF DMA so member A reads its lhsT
        # (K^T) from partitions 0-63 and member B its rhs (Q^T) from
        # 64-127.
        qka = qkpool.tile([P, LQ], BF16)
        qkb = qkpool.tile([P, LQ], BF16)
        # dispatch all input DMAs first so their transfers overlap; the
        # dependent dup DMAs follow (a sem-wait at the dup's queue slot
        # doesn't delay transfers already in flight)
        sts = []
        for c in range(4):
            st = stage.tile([P, 512], F32, tag="st", bufs=4, name=f"st{c}")
            nc.sync.dma_start(out=st, in_=qk_ap[:, 512 * c:512 * c + 512])
            sts.append(st)
        # v_ap [128, 16, 64] f32, already host-shuffled so that
        # v_stage[p, t, :] = v row (t*128 + p): tile t = k-rows
        # 128t..128t+127 on partitions, ready as O-matmul lhsT.
        v_stage = stage.tile([P, NT, D], F32)
        nc.sync.dma_start(out=v_stage[:, 0:8, :], in_=v_ap[:, 0:8, :])
        nc.sync.dma_start(out=v_stage[:, 8:16, :], in_=v_ap[:, 8:16, :])
        for c in range(4):
            sl = slice(512 * c, 512 * c + 512)
            nc.vector.tensor_copy(qka[:, sl], sts[c])
            nc.sync.dma_start(out=qkb[0:D, sl], in_=qka[D:P, sl])
            nc.sync.dma_start(out=qkb[D:P, sl], in_=qka[0:D, sl])

        # ---- S matmul rounds ------------------------------------------
        # round r = (pair p = r//4, half h = (r%4)//2, member m = r%2)
        # S^T tile [128 k-rows, 1024 q-cols] for k-tile 2p+m, q-half h.
        def s_round(r):
            p, h, m = r // 4, (r % 4) // 2, r % 2
            kt = 2 * p + m
            sps = psum_s.tile([P, 1024], F32, tag="s", bufs=3, name=f"sps{r}")
            if m == 0:   # A: PE rows 0-63
                lhsT = qkb[0:D, kt * P:(kt + 1) * P]
                rhs_src, r0, r1 = qka, 0, D
            else:        # B: PE rows 64-127
                lhsT = qka[D:P, kt * P:(kt + 1) * P]
                rhs_src, r0, r1 = qkb, D, P
            with tc.high_priority(offset=25):
                for n in range(2):
                    q0 = h * 1024 + n * 512
                    nc.tensor.matmul(
                        sps[:, n * 512:(n + 1) * 512],
                        lhsT=lhsT,
                        rhs=rhs_src[r0:r1, q0:q0 + 512],
                        start=True,
                        stop=True,
                    )
            return sps

        # O^T accumulators: both members accumulate into the same rows
        # (the A+B sum is free PSUM accumulation).  Chunk n = q-cols
        # [512n, 512n+512): chunks 0/1 share o_ps01 (partitions 0-63 /
        # 64-127), chunks 2/3 share o_ps23.
        o_ps01 = psum_o.tile([P, 512], F32, tag="o01", name="ops01")
        o_ps23 = psum_o.tile([P, 512], F32, tag="o23", name="ops23")

        def o_half(n):
            t = o_ps01 if n < 2 else o_ps23
            return t[0:D, :] if n % 2 == 0 else t[D:P, :]

        pend = [s_round(r) for r in range(3)]
        for p in range(NP):
            e_ab = [epool.tile([P, LQ], BF16, tag="e", name=f"e{p}_{m}")
                    for m in range(2)]
            v_scs = [None, None]
            sh0 = [None, None]
            for idx in range(4):
                h, m = idx // 2, idx % 2
                r = 4 * p + idx
                sps = pend.pop(0)
                esl = e_ab[m][:, h * 1024:(h + 1) * 1024]
                if h == 0:
                    # h=0 half-sum rides the exp for free-ish (+187ns
                    # accumulator read on ACT); h=1 goes to DVE so the
                    # two engines split the softmax-denominator work
                    shm = small.tile([P, 1], F32, tag="sh", bufs=16,
                                     name=f"sh{r}")
                    nc.scalar.activation(
                        out=esl, in_=sps, func=Exp, scale=0.125,
                        accum_out=shm,
                    )
                    sh0[m] = shm
                else:
                    nc.scalar.activation(
                        out=esl, in_=sps, func=Exp, scale=0.125,
                    )
                if r + 3 < NR:
                    pend.append(s_round(r + 3))
                if h == 1:
                    sh1 = small.tile([P, 1], F32, tag="sh1", bufs=16,
                                     name=f"sg{r}")
                    nc.vector.tensor_reduce(sh1, esl, axis=AxX, op=Add)
                    stot = small.tile([P, 1], F32, tag="stot", bufs=16,
                                      name=f"st{p}_{m}")
                    nc.vector.tensor_add(stot, sh0[m], sh1)
                    rec = small.tile([P, 1], F32, tag="rec", bufs=16,
                                     name=f"rc{p}_{m}")
                    nc.vector.reciprocal(rec, stot)
                    v_sc = vpool.tile([P, D], BF16, tag="vsc",
                                      name=f"vs{p}_{m}")
                    nc.vector.tensor_scalar_mul(
                        v_sc, v_stage[:, 2 * p + m, :], rec)
                    v_scs[m] = v_sc
            # 8 O matmuls per pair; consecutive ones alternate column
            # quadrants (chunk parity) so they overlap on the PE.
            for nb in range(2):          # chunk block: (0,1) then (2,3)
                for m in range(2):
                    for n in (2 * nb, 2 * nb + 1):
                        nc.tensor.matmul(
                            o_half(n),
                            lhsT=v_scs[m],
                            rhs=e_ab[m][:, n * 512:(n + 1) * 512],
                            start=(p == 0 and m == 0),
                            stop=(p == NP - 1 and m == 1),
                        )

        # ---- epilogue: cast O^T chunks to bf16, DMA out as [64, 2048] -
        # ACT is idle after the last exp; DVE still drains the last Z
        # chain.  Split the 4 cast-copies across both.
        obuf = qkpool.tile([D, LQ], BF16)
        for n in range(4):
            sl = slice(512 * n, 512 * n + 512)
            eng = nc.scalar.copy if n % 2 == 0 else nc.vector.tensor_copy
            eng(obuf[:, sl], o_half(n))
        nc.sync.dma_start(out=o_ap[:, 0:1024], in_=obuf[:, 0:1024])
        nc.sync.dma_start(out=o_ap[:, 1024:2048], in_=obuf[:, 1024:2048])


_CACHED = {}


def _build():
    if "nc" in _CACHED:
        return _CACHED["nc"]
    nc = bacc.Bacc("TRN2", target_bir_lowering=False, debug=False)
    qk = nc.dram_tensor("qk", [P, LQ], F32, kind="ExternalInput")
    v = nc.dram_tensor("v", [P, NT, D], F32, kind="ExternalInput")
    o = nc.dram_tensor("o", [D, LQ], BF16, kind="ExternalOutput")
    with tile.TileContext(nc) as tc:
        _emit(tc, o[:], qk[:], v[:])
    nc.finalize()
    _CACHED["nc"] = nc
    return nc


def kernel(query, key, value, _trace=False, _trace_kwargs=None):
    query = np.asarray(query, dtype=np.float32)
    key = np.asarray(key, dtype=np.float32)
    value = np.asarray(value, dtype=np.float32)
    assert query.shape == (B, LQ, D), query.shape
    nc = _build()
    in_maps = []
    for i in range(B):
        qk = np.empty((P, LQ), dtype=np.float32)
        qk[0:D] = query[i].T
        qk[D:P] = key[i].T
        vsh = np.ascontiguousarray(
            value[i].reshape(NT, P, D).transpose(1, 0, 2))
        in_maps.append({"qk": qk, "v": vsh})
    kwargs = {}
    if _trace:
        kwargs["trace"] = True
        kwargs.update(_trace_kwargs or {})
    res = run_bass_kernel_spmd(nc, in_maps, core_ids=list(range(B)), **kwargs)
    out = np.stack([
        np.asarray(res.results[i]["o"]).astype(np.float32).T
        for i in range(B)
    ])
    if _trace:
        return out, res
    return out


if __name__ == "__main__":
    rng = np.random.default_rng(0)
    q = rng.standard_normal((B, LQ, D), dtype=np.float32)
    k = rng.standard_normal((B, LQ, D), dtype=np.float32)
    v = rng.standard_normal((B, LQ, D), dtype=np.float32)
    o = kernel(q, k, v)
    print(o.shape, o.dtype)


# revision 13
# speedup vs baseline: 1.1755x; 1.0448x over previous
"""Trainium2 Bass kernel for nn_DotProductAttention (softmax over QUERY axis).

reference:
    scores  = einsum("bqd,bkd->bqk", q, k) / sqrt(d)      # [B, Lq, Lk]
    weights = softmax(scores, axis=1)                     # over q (axis 1!)
    out     = einsum("bqk,bkd->bqd", weights, v)          # [B, Lq, d]

Sharding: data-parallel over batch, one batch element per NeuronCore (B=8).

Host-side work is layout-only (slicing/transpose/reshape for the chosen
sharding): each core receives qk = [q_i^T ; k_i^T] stacked [128, 2048] f32
and v pre-shuffled to [128, 16, 64] (partition = k % 128).  The core
returns O^T [64, 2048] bf16 which the host transposes back.  All
arithmetic (casts, matmuls, softmax) runs on device.

Per-core algorithm (Lq=Lk=2048, d=64):
  - The scalar (ACT) engine is the hard floor: 4.2M exps must flow
    through it (~35us).  It therefore runs ONLY the 32 [128,1024] exp
    instructions; everything else lives on other engines.
  - S^T[k, q] per k-tile via PE: lhsT = K^T tile, rhs = Q^T, contract
    over d=64.  Even k-tiles (member A) use PE rows 0-63, odd (B) rows
    64-127, so consecutive matmuls overlap on disjoint quadrants.  Both
    operand layouts come straight from SBUF: QKa = [Q^T ; K^T] stacked on
    partitions (cast from the DMAed f32), QKb = [K^T ; Q^T] duplicated
    via SBUF->SBUF DMA (no PE transposes, no PSUM staging for inputs).
  - S PSUM is a 3-deep ring of [128,1024] tiles (6 banks): the S matmuls
    for round r+3 only wait on exp(r), giving the PE ~2 exp-times of
    slack so ACT never starves.
  - Softmax denominators: DVE tensor_reduce over the bf16 E slices,
    then add/reciprocal/scale-V (all DVE).  1/Z folds into V rows.
  - O^T[d, q] accumulates in PSUM over all 16 k-tiles AND both members
    (A+B sum falls out of PSUM accumulation for free).  Two [128,512]
    tiles (2 banks): q-chunks 0/2 on partitions 0-63, 1/3 on 64-127.
    Consecutive O matmuls alternate chunk column-quadrants.
  - Epilogue: 4 cast-copies (PSUM f32 -> bf16, split ACT/DVE) + 2 DMAs.

No max-subtraction in softmax: scores ~ N(0,1), max over 2048 ~ 5; exp
never overflows and fp32 exp is exact to ~2 ULP here.
"""

import contextlib
import os
import sys

for _p in ("/opt/trn_rl_repo", "/root/.axon_site/_ro/trn_rl_repo"):
    if os.path.isdir(_p) and _p not in sys.path:
        sys.path.append(_p)

import numpy as np

import concourse.bacc as bacc
import concourse.bass as bass
import concourse.mybir as mybir
import concourse.tile as tile
from concourse.bass_utils import run_bass_kernel_spmd

B, LQ, LK, D = 8, 2048, 2048, 64
P = 128                  # partitions
NT = LK // P             # 16 k-tiles
NP = NT // 2             # 8 k-tile pairs (A = even tile, B = odd tile)
NR = 4 * NP              # 32 rounds; round = one [128,1024] exp
F32 = mybir.dt.float32
BF16 = mybir.dt.bfloat16


def _emit(tc: tile.TileContext, o_ap, qk_ap, qk2_ap, v_ap):
    nc = tc.nc
    Exp = mybir.ActivationFunctionType.Exp
    AxX = mybir.AxisListType.X
    Add = mybir.AluOpType.add

    with contextlib.ExitStack() as ctx:
        consts = ctx.enter_context(tc.tile_pool(name="consts", bufs=1))
        stage = ctx.enter_context(tc.tile_pool(name="stage", bufs=1))
        qkpool = ctx.enter_context(tc.tile_pool(name="qkpool", bufs=1))
        epool = ctx.enter_context(tc.tile_pool(name="epool", bufs=6))
        vpool = ctx.enter_context(tc.tile_pool(name="vpool", bufs=6))
        small = ctx.enter_context(tc.tile_pool(name="small", bufs=32))
        psum_s = ctx.enter_context(
            tc.tile_pool(name="psum_s", bufs=3, space=bass.MemorySpace.PSUM)
        )
        psum_o = ctx.enter_context(
            tc.tile_pool(name="psum_o", bufs=1, space=bass.MemorySpace.PSUM)
        )

        # ---- preload the Exp activation table while DMAs run ----------
        dummy = consts.tile([P, 1], F32)
        nc.gpsimd.memset(dummy, 0.0)
        dume = consts.tile([P, 1], F32)
        nc.scalar.activation(out=dume, in_=dummy, func=Exp)

        # ---- input staging --------------------------------------------
        # qk_ap [128, 2048] f32: partitions 0-63 = Q^T (d-major), 64-127
        # = K^T.  Cast to bf16 into QKa; QKb = partition-swapped copy
        # ([K^T ; Q^T]) via SBUF->SBUF DMA so member A reads its lhsT
        # (K^T) from partitions 0-63 and member B its rhs (Q^T) from
        # 64-127.
        qka = qkpool.tile([P, LQ], BF16)
        qkb = qkpool.tile([P, LQ], BF16)
        # qk_ap/qk2_ap are the host-stacked [Q^T ; K^T] and [K^T ; Q^T]
        # layouts.  Both HWDGE queues (SP + ACT, idle before the first
        # exp) dispatch in parallel; V goes via the gpsimd SWDGE.
        sta, stb = [], []
        for c in range(4):
            sl = slice(512 * c, 512 * c + 512)
            st_a = stage.tile([P, 512], F32, tag="sta", bufs=4, name=f"sa{c}")
            nc.sync.dma_start(out=st_a, in_=qk_ap[:, sl])
            sta.append(st_a)
            st_b = stage.tile([P, 512], F32, tag="stb", bufs=4, name=f"sb{c}")
            nc.scalar.dma_start(out=st_b, in_=qk2_ap[:, sl])
            stb.append(st_b)
        # v_ap [128, 16, 64] f32, already host-shuffled so that
        # v_stage[p, t, :] = v row (t*128 + p): tile t = k-rows
        # 128t..128t+127 on partitions, ready as O-matmul lhsT.
        v_stage = stage.tile([P, NT, D], F32)
        nc.gpsimd.dma_start(out=v_stage[:, 0:8, :], in_=v_ap[:, 0:8, :])
        nc.gpsimd.dma_start(out=v_stage[:, 8:16, :], in_=v_ap[:, 8:16, :])
        for c in range(4):
            sl = slice(512 * c, 512 * c + 512)
            nc.vector.tensor_copy(qka[:, sl], sta[c])
            nc.vector.tensor_copy(qkb[:, sl], stb[c])

        # ---- S matmul rounds ------------------------------------------
        # round r = (pair p = r//4, half h = (r%4)//2, member m = r%2)
        # S^T tile [128 k-rows, 1024 q-cols] for k-tile 2p+m, q-half h.
        def s_round(r):
            p, h, m = r // 4, (r % 4) // 2, r % 2
            kt = 2 * p + m
            sps = psum_s.tile([P, 1024], F32, tag="s", bufs=3, name=f"sps{r}")
            if m == 0:   # A: PE rows 0-63
                lhsT = qkb[0:D, kt * P:(kt + 1) * P]
                rhs_src, r0, r1 = qka, 0, D
            else:        # B: PE rows 64-127
                lhsT = qka[D:P, kt * P:(kt + 1) * P]
                rhs_src, r0, r1 = qkb, D, P
            with tc.high_priority(offset=25):
                for n in range(2):
                    q0 = h * 1024 + n * 512
                    nc.tensor.matmul(
                        sps[:, n * 512:(n + 1) * 512],
                        lhsT=lhsT,
                        rhs=rhs_src[r0:r1, q0:q0 + 512],
                        start=True,
                        stop=True,
                    )
            return sps

        # O^T accumulators: both members accumulate into the same rows
        # (the A+B sum is free PSUM accumulation).  Chunk n = q-cols
        # [512n, 512n+512): chunks 0/1 share o_ps01 (partitions 0-63 /
        # 64-127), chunks 2/3 share o_ps23.
        o_ps01 = psum_o.tile([P, 512], F32, tag="o01", name="ops01")
        o_ps23 = psum_o.tile([P, 512], F32, tag="o23", name="ops23")

        def o_half(n):
            t = o_ps01 if n < 2 else o_ps23
            return t[0:D, :] if n % 2 == 0 else t[D:P, :]

        pend = [s_round(r) for r in range(3)]
        for p in range(NP):
            e_ab = [epool.tile([P, LQ], BF16, tag="e", name=f"e{p}_{m}")
                    for m in range(2)]
            v_scs = [None, None]
            sh0 = [None, None]
            for idx in range(4):
                h, m = idx // 2, idx % 2
                r = 4 * p + idx
                sps = pend.pop(0)
                esl = e_ab[m][:, h * 1024:(h + 1) * 1024]
                if h == 0:
                    # h=0 half-sum rides the exp for free-ish (+187ns
                    # accumulator read on ACT); h=1 goes to DVE so the
                    # two engines split the softmax-denominator work
                    shm = small.tile([P, 1], F32, tag="sh", bufs=16,
                                     name=f"sh{r}")
                    nc.scalar.activation(
                        out=esl, in_=sps, func=Exp, scale=0.125,
                        accum_out=shm,
                    )
                    sh0[m] = shm
                else:
                    nc.scalar.activation(
                        out=esl, in_=sps, func=Exp, scale=0.125,
                    )
                if r + 3 < NR:
                    pend.append(s_round(r + 3))
                if h == 1:
                    sh1 = small.tile([P, 1], F32, tag="sh1", bufs=16,
                                     name=f"sg{r}")
                    nc.vector.tensor_reduce(sh1, esl, axis=AxX, op=Add)
                    stot = small.tile([P, 1], F32, tag="stot", bufs=16,
                                      name=f"st{p}_{m}")
                    nc.vector.tensor_add(stot, sh0[m], sh1)
                    rec = small.tile([P, 1], F32, tag="rec", bufs=16,
                                     name=f"rc{p}_{m}")
                    nc.vector.reciprocal(rec, stot)
                    v_sc = vpool.tile([P, D], BF16, tag="vsc",
                                      name=f"vs{p}_{m}")
                    nc.vector.tensor_scalar_mul(
                        v_sc, v_stage[:, 2 * p + m, :], rec)
                    v_scs[m] = v_sc
            # 8 O matmuls per pair; consecutive ones alternate column
            # quadrants (chunk parity) so they overlap on the PE.
            for nb in range(2):          # chunk block: (0,1) then (2,3)
                for m in range(2):
                    for n in (2 * nb, 2 * nb + 1):
                        nc.tensor.matmul(
                            o_half(n),
                            lhsT=v_scs[m],
                            rhs=e_ab[m][:, n * 512:(n + 1) * 512],
                            start=(p == 0 and m == 0),
                            stop=(p == NP - 1 and m == 1),
                        )

        # ---- epilogue: cast O^T chunks to bf16, DMA out as [64, 2048] -
        # ACT is idle after the last exp; DVE still drains the last Z
        # chain.  Split the 4 cast-copies across both.
        obuf = qkpool.tile([D, LQ], BF16)
        for n in range(4):
            sl = slice(512 * n, 512 * n + 512)
            eng = nc.scalar.copy if n % 2 == 0 else nc.vector.tensor_copy
            eng(obuf[:, sl], o_half(n))
        nc.sync.dma_start(out=o_ap[:, 0:1024], in_=obuf[:, 0:1024])
        nc.sync.dma_start(out=o_ap[:, 1024:2048], in_=obuf[:, 1024:2048])


_CACHED = {}


def _build():
    if "nc" in _CACHED:
        return _CACHED["nc"]
    nc = bacc.Bacc("TRN2", target_bir_lowering=False, debug=False)
    qk = nc.dram_tensor("qk", [P, LQ], F32, kind="ExternalInput")
    qk2 = nc.dram_tensor("qk2", [P, LQ], F32, kind="ExternalInput")
    v = nc.dram_tensor("v", [P, NT, D], F32, kind="ExternalInput")
    o = nc.dram_tensor("o", [D, LQ], BF16, kind="ExternalOutput")
    with tile.TileContext(nc) as tc:
        _emit(tc, o[:], qk[:], qk2[:], v[:])
    nc.finalize()
    _CACHED["nc"] = nc
    return nc


def kernel(query, key, value, _trace=False, _trace_kwargs=None):
    query = np.asarray(query, dtype=np.float32)
    key = np.asarray(key, dtype=np.float32)
    value = np.asarray(value, dtype=np.float32)
    assert query.shape == (B, LQ, D), query.shape
    nc = _build()
    in_maps = []
    for i in range(B):
        qk = np.empty((P, LQ), dtype=np.float32)
        qk[0:D] = query[i].T
        qk[D:P] = key[i].T
        qk2 = np.empty((P, LQ), dtype=np.float32)
        qk2[0:D] = qk[D:P]
        qk2[D:P] = qk[0:D]
        vsh = np.ascontiguousarray(
            value[i].reshape(NT, P, D).transpose(1, 0, 2))
        in_maps.append({"qk": qk, "qk2": qk2, "v": vsh})
    kwargs = {}
    if _trace:
        kwargs["trace"] = True
        kwargs.update(_trace_kwargs or {})
    res = run_bass_kernel_spmd(nc, in_maps, core_ids=list(range(B)), **kwargs)
    out = np.stack([
        np.asarray(res.results[i]["o"]).astype(np.float32).T
        for i in range(B)
    ])
    if _trace:
        return out, res
    return out


if __name__ == "__main__":
    rng = np.random.default_rng(0)
    q = rng.standard_normal((B, LQ, D), dtype=np.float32)
    k = rng.standard_normal((B, LQ, D), dtype=np.float32)
    v = rng.standard_normal((B, LQ, D), dtype=np.float32)
    o = kernel(q, k, v)
    print(o.shape, o.dtype)
